# revision 26
# baseline (speedup 1.0000x reference)
"""MultiHeadAttention Trainium2 kernel (8-core sharded).

Reference computation (per batch b):
  qh = einsum('sd,hdk->hsk', q[b], Wq) + bq   (same k, v)
  scores = qh @ kh^T / sqrt(64); weights = softmax(scores)
  attn = weights @ vh; out = concat_heads(attn) @ Wo + bo
Returns (out [B,S,D], weights [B,H,S,S]).

Sharding: core c handles batch b = c//4 and heads [4g, 4g+4), g = c%4.
Each core computes its 4 heads' weights plus a partial output projection
(contracting only its heads' slice of Wo); the host sums the 4 partials
per batch and adds the bias terms (bv folds into a constant row because
softmax rows sum to 1: attn = attn_nobv + bv, so out gains bv_flat@Wo).
The host ships q/k/v pre-transposed ([D, S]) so the device needs no PE
transposes for the projections.

Device pipeline per core:
  1. projections on PE from qT/kT/vT chunks (fp32r):
     qhT/khT [dk-on-partitions, s] fp32r (+ bf16 copies), vh bf16.
  2. Per head:
     (a) scoresT chunks (PE, bf16) -> exp (ACT) -> expT bf16
     (b) scores chunks (PE, fp32r) -> exp+rowsum (ACT) -> normalize
         (GpSimd normalize_recip; denominators become reciprocals
         in-place) -> weights DMA (fp32, accurate path)
     (c) attn: attnT_un = vh^T @ expT (PE, bf16); normalized along q
         using the (b) reciprocals: PE-mini-transpose -> DRAM bounce ->
         partition-broadcast load (GpSimd, cast bf16) -> DVE multiply.
  3. output projection (PE, bf16) -> partial out DMA (fp32).
"""

import numpy as np
import ml_dtypes

B, S, DIM, H, DK, DV = 2, 2048, 1024, 16, 64, 64
HPC = 4                     # heads per core
NCORES = 8
SCALE = 1.0 / np.sqrt(DV)   # folded into Wq/bq on host
NT = S // 128               # 16 s-tiles
NJ = DIM // 128             # 8 d-chunks

_cache = {}


def _build():
    import concourse.bass as bass
    import concourse.mybir as mybir
    import concourse.tile as tile
    from concourse import bacc
    from contextlib import ExitStack

    f32 = mybir.dt.float32
    f32r = mybir.dt.float32r
    bf16 = mybir.dt.bfloat16
    ACT_EXP = mybir.ActivationFunctionType.Exp

    nc = bacc.Bacc(None, target_bir_lowering=False)

    qt_d = nc.dram_tensor("qT", [DIM, S], bf16, kind="ExternalInput")
    kt_d = nc.dram_tensor("kT", [DIM, S], bf16, kind="ExternalInput")
    vt_d = nc.dram_tensor("vT", [DIM, S], bf16, kind="ExternalInput")
    wq_d = nc.dram_tensor("wq", [DIM, 256], bf16, kind="ExternalInput")
    wk_d = nc.dram_tensor("wk", [DIM, 256], bf16, kind="ExternalInput")
    wv_d = nc.dram_tensor("wv", [DIM, 256], bf16, kind="ExternalInput")
    bq_d = nc.dram_tensor("bq", [128, 2], f32, kind="ExternalInput")
    bk_d = nc.dram_tensor("bk", [128, 2], f32, kind="ExternalInput")
    wo_d = nc.dram_tensor("wo", [256, DIM], bf16, kind="ExternalInput")
    id_d = nc.dram_tensor("ident", [128, 128], f32, kind="ExternalInput")
    w_out = nc.dram_tensor("w_out", [HPC, S, S], f32, kind="ExternalOutput")
    o_out = nc.dram_tensor("o_out", [S, DIM], f32, kind="ExternalOutput")

    with tile.TileContext(nc) as tc, ExitStack() as ctx:
        singles = ctx.enter_context(tc.tile_pool(name="singles", bufs=1))
        persist = ctx.enter_context(tc.tile_pool(name="persist", bufs=1))
        qtp = ctx.enter_context(tc.tile_pool(name="qtp", bufs=1))
        wstp2 = ctx.enter_context(tc.tile_pool(name="wstp2", bufs=3))
        ostp = ctx.enter_context(tc.tile_pool(name="ostp", bufs=2))
        small = ctx.enter_context(tc.tile_pool(name="small", bufs=2))
        stage = ctx.enter_context(tc.tile_pool(name="stage", bufs=4))
        bcp = ctx.enter_context(tc.tile_pool(name="bcp", bufs=1))
        psum = ctx.enter_context(tc.tile_pool(name="ps", bufs=3, space="PSUM"))
        psums = ctx.enter_context(tc.tile_pool(name="pss", bufs=2, space="PSUM"))
        dram = ctx.enter_context(tc.tile_pool(name="drs", bufs=2, space="DRAM"))

        # ---- constants ----
        ident = singles.tile([128, 128], f32, tag="ident")
        nc.sync.dma_start(out=ident, in_=id_d[:, :])
        wq_sb = singles.tile([128, NJ, 256], bf16, tag="wq")
        wk_sb = singles.tile([128, NJ, 256], bf16, tag="wk")
        wv_sb = singles.tile([128, NJ, 256], bf16, tag="wv")
        nc.sync.dma_start(out=wq_sb, in_=wq_d.rearrange("(j p) c -> p j c", p=128))
        nc.sync.dma_start(out=wk_sb, in_=wk_d.rearrange("(j p) c -> p j c", p=128))
        nc.sync.dma_start(out=wv_sb, in_=wv_d.rearrange("(j p) c -> p j c", p=128))
        wo_sb = singles.tile([128, 2, DIM], bf16, tag="wo")
        nc.sync.dma_start(out=wo_sb, in_=wo_d.rearrange("(m p) c -> p m c", p=128))
        bq_sb = singles.tile([128, 2], f32, tag="bq")
        bk_sb = singles.tile([128, 2], f32, tag="bk")
        nc.sync.dma_start(out=bq_sb, in_=bq_d[:, :])
        nc.sync.dma_start(out=bk_sb, in_=bk_d[:, :])

        # ---- persistent activations ----
        qhT = persist.tile([128, 2, S], bf16, tag="qhT")   # [(h%2)*64+dk, h//2, s]
        khT = persist.tile([128, 2, S], bf16, tag="khT")
        vha = persist.tile([128, NT, HPC, 64], bf16, tag="vha")  # [k%128, kt, h, dv]
        expT = persist.tile([128, NT, S], bf16, tag="expT")      # per-head reuse
        attnT = persist.tile([128, 2, S], bf16, tag="attnT")     # [(h%2)*64+dv, h//2, q]

        # ---- phase 1: load transposed chunks, project ----
        def load_project(src_d, which, sc):
            """One 512-col slab of qT/kT/vT: load, project."""
            xt = qtp.tile([128, NJ, 512], bf16, tag="xt")  # [d%128, j, s-in-slab]
            nc.sync.dma_start(
                out=xt,
                in_=src_d.rearrange("(j p) s -> p j s", p=128)[
                    :, :, sc * 512:(sc + 1) * 512])
            if which == "q" or which == "k":
                w_sb = wq_sb if which == "q" else wk_sb
                b_sb = bq_sb if which == "q" else bk_sb
                dstT = qhT if which == "q" else khT
                for m in range(2):
                    pp = psums.tile([128, 512], f32, tag="sm")
                    for j in range(NJ):
                        nc.tensor.matmul(pp, w_sb[:, j, m * 128:(m + 1) * 128],
                                         xt[:, j, :],
                                         start=(j == 0), stop=(j == NJ - 1))
                    with nc.allow_low_precision(reason="bf16 activations"):
                        nc.vector.tensor_scalar_add(
                            dstT[:, m, sc * 512:(sc + 1) * 512], pp,
                            b_sb[:, m:m + 1])
            else:
                for st4 in range(4):
                    pp = psums.tile([128, 256], f32, tag="sm")
                    for j in range(NJ):
                        nc.tensor.matmul(pp, xt[:, j, st4 * 128:(st4 + 1) * 128],
                                         wv_sb[:, j, :],
                                         start=(j == 0), stop=(j == NJ - 1))
                    kt = sc * 4 + st4
                    nc.vector.tensor_copy(
                        out=vha[:, kt, :, :],
                        in_=pp.rearrange("p (h d) -> p h d", h=HPC))

        for sc in range(4):
            load_project(qt_d, "q", sc)
            load_project(kt_d, "k", sc)
            load_project(vt_d, "v", sc)

        # ---- phase 2: paired heads ----
        # Each stream-phase interleaves (a)-scores of one head with
        # (b)-scores of the opposite-parity head: lhsT row-groups alternate
        # (0-63 vs 64-127) so LDWEIGHTS pulls ahead and the PE streams,
        # and ACT always has (b)-work while (a) waits on expT reuse.
        rshs = {}
        stgs = {}

        def a_chunk(h, t):
            p0, hp = (h % 2) * 64, h // 2
            for ah in range(2):
                psA = psum.tile([128, 1024], f32, tag="med")
                for qc in range(2):
                    q0 = ah * 2 + qc
                    nc.tensor.matmul(
                        psA[:, qc * 512:(qc + 1) * 512],
                        khT[p0:p0 + 64, hp, t * 128:(t + 1) * 128],
                        qhT[p0:p0 + 64, hp, q0 * 512:(q0 + 1) * 512],
                        start=True, stop=True)
                nc.scalar.activation(
                    out=expT[:, t, ah * 1024:(ah + 1) * 1024], in_=psA,
                    func=ACT_EXP)

        def b_chunk(h, t):
            p0, hp = (h % 2) * 64, h // 2
            rsh = rshs[h]
            rsp = small.tile([128, 2], f32, tag="rsp")
            wnorm = wstp2.tile([128, 2048], f32, tag="wnormbig")
            wraw = wstp2.tile([128, 1024], f32, tag="wraw")
            for bh in range(2):
                psB = psum.tile([128, 1024], f32, tag="med")
                for kc in range(2):
                    k0 = bh * 2 + kc
                    nc.tensor.matmul(
                        psB[:, kc * 512:(kc + 1) * 512],
                        qhT[p0:p0 + 64, hp, t * 128:(t + 1) * 128],
                        khT[p0:p0 + 64, hp, k0 * 512:(k0 + 1) * 512],
                        start=True, stop=True)
                nc.scalar.activation(
                    out=wraw if bh == 0 else wnorm[:, 1024:2048],
                    in_=psB, func=ACT_EXP, accum_out=rsp[:, bh:bh + 1])
            # total rowsum; normalize_recip writes back its reciprocal
            nc.vector.tensor_add(rsh[:, t:t + 1], rsp[:, 0:1], rsp[:, 1:2])
            nc.gpsimd.normalize_recip(wnorm[:, 0:1024], wraw, rsh[:, t:t + 1])
            with nc.allow_low_precision(reason="bf16 path"):
                nc.vector.tensor_scalar_mul(wnorm[:, 1024:2048],
                                            wnorm[:, 1024:2048],
                                            rsh[:, t:t + 1])
            nc.sync.dma_start(out=w_out[h, t * 128:(t + 1) * 128, :], in_=wnorm)

        def ab_chunk(ha, hb, t):
            """One q/k-block step for both streams, matmuls interleaved so
            consecutive PE ops alternate row-groups (64-partition heads of
            opposite parity) and LDWEIGHTS can pull ahead."""
            pa, hpa = (ha % 2) * 64, ha // 2
            pb, hpb = (hb % 2) * 64, hb // 2
            rsh = rshs[hb]
            rsp = small.tile([128, 2], f32, tag="rsp")
            wnorm = wstp2.tile([128, 2048], f32, tag="wnormbig")
            wraw = wstp2.tile([128, 1024], f32, tag="wraw")
            for half in range(2):
                psA = psum.tile([128, 1024], f32, tag="med")
                psB = psum.tile([128, 1024], f32, tag="med")
                for c in range(2):
                    q0 = half * 2 + c
                    nc.tensor.matmul(
                        psA[:, c * 512:(c + 1) * 512],
                        khT[pa:pa + 64, hpa, t * 128:(t + 1) * 128],
                        qhT[pa:pa + 64, hpa, q0 * 512:(q0 + 1) * 512],
                        start=True, stop=True)
                    nc.tensor.matmul(
                        psB[:, c * 512:(c + 1) * 512],
                        qhT[pb:pb + 64, hpb, t * 128:(t + 1) * 128],
                        khT[pb:pb + 64, hpb, q0 * 512:(q0 + 1) * 512],
                        start=True, stop=True)
                nc.scalar.activation(
                    out=expT[:, t, half * 1024:(half + 1) * 1024], in_=psA,
                    func=ACT_EXP)
                nc.scalar.activation(
                    out=wraw if half == 0 else wnorm[:, 1024:2048],
                    in_=psB, func=ACT_EXP, accum_out=rsp[:, half:half + 1])
            nc.vector.tensor_add(rsh[:, t:t + 1], rsp[:, 0:1], rsp[:, 1:2])
            nc.gpsimd.normalize_recip(wnorm[:, 0:1024], wraw, rsh[:, t:t + 1])
            with nc.allow_low_precision(reason="bf16 path"):
                nc.vector.tensor_scalar_mul(wnorm[:, 1024:2048],
                                            wnorm[:, 1024:2048],
                                            rsh[:, t:t + 1])
            nc.sync.dma_start(out=w_out[hb, t * 128:(t + 1) * 128, :], in_=wnorm)

        def stream_phase(ha, hb):
            rshs[hb] = small.tile([128, NT], f32, tag="rsh", name="rsh")
            for t in range(NT):
                ab_chunk(ha, hb, t)

        def attn_mms(h):
            out = []
            for qt in range(4):
                psAt = psums.tile([64, 512], f32, tag="sm")
                for kt in range(NT):
                    nc.tensor.matmul(
                        psAt, vha[:, kt, h, :],
                        expT[:, kt, qt * 512:(qt + 1) * 512],
                        start=(kt == 0), stop=(kt == NT - 1))
                stg = stage.tile([64, 512], bf16, tag="stg")
                nc.vector.tensor_copy(out=stg, in_=psAt)
                out.append(stg)
            stgs[h] = out

        def attn_norm(h):
            # broadcast 1/rowsum along q (free axis): transpose rsh on PE,
            # bounce through DRAM, partition-broadcast load with bf16 cast
            p0, hp = (h % 2) * 64, h // 2
            psr = psums.tile([NT, 128], f32, tag="sm")
            nc.tensor.transpose(psr, rshs[h], ident)
            rcT = small.tile([NT, 128], f32, tag="rcT")
            nc.vector.tensor_copy(out=rcT, in_=psr)
            drh = dram.tile([NT, 128], f32, tag="drh")
            nc.sync.dma_start(out=drh, in_=rcT)
            bc = bcp.tile([64, S], bf16, tag="bc")
            nc.gpsimd.dma_start(
                out=bc,
                in_=drh.rearrange("t (o p) -> o (t p)", o=1).to_broadcast([64, S]))
            for qt in range(4):
                nc.vector.tensor_mul(
                    attnT[p0:p0 + 64, hp, qt * 512:(qt + 1) * 512],
                    stgs[h][qt], bc[:, qt * 512:(qt + 1) * 512])

        stream_phase(0, 1)   # expT_0, rsh_1
        attn_mms(0)          # runs alongside next phase
        stream_phase(1, 0)   # expT_1 (waits on attn_0 reads), rsh_0
        attn_norm(0)
        attn_mms(1)
        attn_norm(1)
        stream_phase(2, 3)
        attn_mms(2)
        stream_phase(3, 2)
        attn_norm(2)
        attn_mms(3)
        attn_norm(3)

        # ---- phase 3: output projection (partial, no bias) ----
        for st in range(NT):
            psO = psum.tile([128, 1024], f32, tag="med")
            for oc in range(2):
                for m in range(2):
                    nc.tensor.matmul(
                        psO[:, oc * 512:(oc + 1) * 512],
                        attnT[:, m, st * 128:(st + 1) * 128],
                        wo_sb[:, m, oc * 512:(oc + 1) * 512],
                        start=(m == 0), stop=(m == 1))
            ost = ostp.tile([128, DIM], f32, tag="ost")
            nc.vector.tensor_copy(out=ost, in_=psO)
            nc.sync.dma_start(out=o_out[st * 128:(st + 1) * 128, :], in_=ost)

    nc.compile()
    return nc


def _prep_core_inputs(c, q, k, v, Wq, bq, Wk, bk, Wv, Wo, ident):
    b, g = divmod(c, HPC)
    hs = g * HPC
    sl = slice(hs, hs + HPC)
    return {
        "qT": np.ascontiguousarray(q[b].T.astype(ml_dtypes.bfloat16)),
        "kT": np.ascontiguousarray(k[b].T.astype(ml_dtypes.bfloat16)),
        "vT": np.ascontiguousarray(v[b].T.astype(ml_dtypes.bfloat16)),
        "wq": np.ascontiguousarray(
            (Wq[sl] * SCALE).transpose(1, 0, 2).reshape(DIM, HPC * DK)
            .astype(ml_dtypes.bfloat16)),
        "wk": np.ascontiguousarray(
            Wk[sl].transpose(1, 0, 2).reshape(DIM, HPC * DK)
            .astype(ml_dtypes.bfloat16)),
        "wv": np.ascontiguousarray(
            Wv[sl].transpose(1, 0, 2).reshape(DIM, HPC * DV)
            .astype(ml_dtypes.bfloat16)),
        "bq": np.ascontiguousarray(
            (bq[sl] * SCALE).reshape(HPC * DK).reshape(2, 128).T.astype(np.float32)),
        "bk": np.ascontiguousarray(
            bk[sl].reshape(HPC * DK).reshape(2, 128).T.astype(np.float32)),
        "wo": np.ascontiguousarray(
            Wo[g * 256:(g + 1) * 256].astype(ml_dtypes.bfloat16)),
        "ident": ident,
    }


def kernel(q, k, v, Wq, bq, Wk, bk, Wv, bv, Wo, bo):
    from concourse.bass_utils import run_bass_kernel_spmd

    q, k, v = (np.asarray(x, np.float32) for x in (q, k, v))
    Wq, bq, Wk, bk, Wv, bv, Wo, bo = (
        np.asarray(x, np.float32) for x in (Wq, bq, Wk, bk, Wv, bv, Wo, bo))

    if "nc" not in _cache:
        _cache["nc"] = _build()
    nc = _cache["nc"]

    ident = np.eye(128, dtype=np.float32)
    in_maps = [_prep_core_inputs(c, q, k, v, Wq, bq, Wk, bk, Wv, Wo, ident)
               for c in range(NCORES)]
    res = run_bass_kernel_spmd(nc, in_maps, core_ids=list(range(NCORES))).results

    weights = np.empty((B, H, S, S), np.float32)
    out = np.empty((B, S, DIM), np.float32)
    extra = (bv.reshape(DIM) @ Wo + bo).astype(np.float32)  # bv folds via softmax sum=1
    for b in range(B):
        acc = None
        for g in range(HPC):
            r = res[b * HPC + g]
            weights[b, g * HPC:(g + 1) * HPC] = r["w_out"]
            acc = r["o_out"] if acc is None else acc + r["o_out"]
        out[b] = acc + extra
    return out, weights


# revision 27
# speedup vs baseline: 1.0095x; 1.0095x over previous
"""MultiHeadAttention Trainium2 kernel (8-core sharded).

Reference computation (per batch b):
  qh = einsum('sd,hdk->hsk', q[b], Wq) + bq   (same k, v)
  scores = qh @ kh^T / sqrt(64); weights = softmax(scores)
  attn = weights @ vh; out = concat_heads(attn) @ Wo + bo
Returns (out [B,S,D], weights [B,H,S,S]).

Sharding: core c handles batch b = c//4 and heads [4g, 4g+4), g = c%4.
Each core computes its 4 heads' weights plus a partial output projection
(contracting only its heads' slice of Wo); the host sums the 4 partials
per batch and adds the bias terms (bv folds into a constant row because
softmax rows sum to 1: attn = attn_nobv + bv, so out gains bv_flat@Wo).
The host ships q/k/v pre-transposed ([D, S]) so the device needs no PE
transposes for the projections.

All matmuls run in bf16 with f32 PSUM accumulation (exp and softmax
normalization stay f32). Device pipeline per core:
  1. projections on PE from bf16 qT/kT/vT chunks:
     qhT/khT [dk-on-partitions, s], vh [k-on-partitions, dv].
  2. Paired stream phases: (a)-orientation scoresT chunks of one head
     interleaved matmul-by-matmul with (b)-orientation scores chunks of
     the opposite-parity head (row-groups alternate, keeping the PE
     array streaming). (a): exp (ACT) -> expT bf16 for attention.
     (b): exp+rowsum (ACT) -> normalize (GpSimd normalize_recip, which
     also writes back 1/rowsum) -> weights DMA (f32).
  3. attn: attnT_un = vh^T @ expT (PE, bf16); normalized along the
     free q axis with the (b) reciprocals via PE-mini-transpose ->
     DRAM bounce -> partition-broadcast load -> DVE multiply.
  4. output projection (PE, bf16) -> partial out DMA (f32).
"""

import numpy as np
import ml_dtypes

B, S, DIM, H, DK, DV = 2, 2048, 1024, 16, 64, 64
HPC = 4                     # heads per core
NCORES = 8
SCALE = 1.0 / np.sqrt(DV)   # folded into Wq/bq on host
NT = S // 128               # 16 s-tiles
NJ = DIM // 128             # 8 d-chunks

_cache = {}


def _build():
    import concourse.bass as bass
    import concourse.mybir as mybir
    import concourse.tile as tile
    from concourse import bacc
    from contextlib import ExitStack

    f32 = mybir.dt.float32
    f32r = mybir.dt.float32r
    bf16 = mybir.dt.bfloat16
    ACT_EXP = mybir.ActivationFunctionType.Exp

    nc = bacc.Bacc(None, target_bir_lowering=False)

    qt_d = nc.dram_tensor("qT", [DIM, S], bf16, kind="ExternalInput")
    kt_d = nc.dram_tensor("kT", [DIM, S], bf16, kind="ExternalInput")
    vt_d = nc.dram_tensor("vT", [DIM, S], bf16, kind="ExternalInput")
    wq_d = nc.dram_tensor("wq", [DIM, 256], bf16, kind="ExternalInput")
    wk_d = nc.dram_tensor("wk", [DIM, 256], bf16, kind="ExternalInput")
    wv_d = nc.dram_tensor("wv", [DIM, 256], bf16, kind="ExternalInput")
    bq_d = nc.dram_tensor("bq", [128, 2], f32, kind="ExternalInput")
    bk_d = nc.dram_tensor("bk", [128, 2], f32, kind="ExternalInput")
    wo_d = nc.dram_tensor("wo", [256, DIM], bf16, kind="ExternalInput")
    id_d = nc.dram_tensor("ident", [128, 128], f32, kind="ExternalInput")
    w_out = nc.dram_tensor("w_out", [HPC, S, S], f32, kind="ExternalOutput")
    o_out = nc.dram_tensor("o_out", [S, DIM], f32, kind="ExternalOutput")

    with tile.TileContext(nc) as tc, ExitStack() as ctx:
        singles = ctx.enter_context(tc.tile_pool(name="singles", bufs=1))
        persist = ctx.enter_context(tc.tile_pool(name="persist", bufs=1))
        qtp = ctx.enter_context(tc.tile_pool(name="qtp", bufs=1))
        wstp2 = ctx.enter_context(tc.tile_pool(name="wstp2", bufs=3))
        ostp = ctx.enter_context(tc.tile_pool(name="ostp", bufs=2))
        small = ctx.enter_context(tc.tile_pool(name="small", bufs=2))
        stage = ctx.enter_context(tc.tile_pool(name="stage", bufs=4))
        bcp = ctx.enter_context(tc.tile_pool(name="bcp", bufs=1))
        psum = ctx.enter_context(tc.tile_pool(name="ps", bufs=3, space="PSUM"))
        psums = ctx.enter_context(tc.tile_pool(name="pss", bufs=2, space="PSUM"))
        dram = ctx.enter_context(tc.tile_pool(name="drs", bufs=2, space="DRAM"))

        # ---- constants ----
        ident = singles.tile([128, 128], f32, tag="ident")
        nc.sync.dma_start(out=ident, in_=id_d[:, :])
        wq_sb = singles.tile([128, NJ, 256], bf16, tag="wq")
        wk_sb = singles.tile([128, NJ, 256], bf16, tag="wk")
        wv_sb = singles.tile([128, NJ, 256], bf16, tag="wv")
        nc.sync.dma_start(out=wq_sb, in_=wq_d.rearrange("(j p) c -> p j c", p=128))
        nc.sync.dma_start(out=wk_sb, in_=wk_d.rearrange("(j p) c -> p j c", p=128))
        nc.sync.dma_start(out=wv_sb, in_=wv_d.rearrange("(j p) c -> p j c", p=128))
        wo_sb = singles.tile([128, 2, DIM], bf16, tag="wo")
        nc.sync.dma_start(out=wo_sb, in_=wo_d.rearrange("(m p) c -> p m c", p=128))
        bq_sb = singles.tile([128, 2], f32, tag="bq")
        bk_sb = singles.tile([128, 2], f32, tag="bk")
        nc.sync.dma_start(out=bq_sb, in_=bq_d[:, :])
        nc.sync.dma_start(out=bk_sb, in_=bk_d[:, :])

        # ---- persistent activations ----
        qhT = persist.tile([128, 2, S], bf16, tag="qhT")   # [(h%2)*64+dk, h//2, s]
        khT = persist.tile([128, 2, S], bf16, tag="khT")
        vha = persist.tile([128, NT, HPC, 64], bf16, tag="vha")  # [k%128, kt, h, dv]
        expT = persist.tile([128, NT, S], bf16, tag="expT")      # per-head reuse
        attnT = persist.tile([128, 2, S], bf16, tag="attnT")     # [(h%2)*64+dv, h//2, q]

        # ---- phase 1: load transposed chunks, project ----
        def load_project(src_d, which, sc):
            """One 512-col slab of qT/kT/vT: load, project."""
            xt = qtp.tile([128, NJ, 512], bf16, tag="xt")  # [d%128, j, s-in-slab]
            nc.sync.dma_start(
                out=xt,
                in_=src_d.rearrange("(j p) s -> p j s", p=128)[
                    :, :, sc * 512:(sc + 1) * 512])
            if which == "q" or which == "k":
                w_sb = wq_sb if which == "q" else wk_sb
                b_sb = bq_sb if which == "q" else bk_sb
                dstT = qhT if which == "q" else khT
                for m in range(2):
                    pp = psums.tile([128, 512], f32, tag="sm")
                    for j in range(NJ):
                        nc.tensor.matmul(pp, w_sb[:, j, m * 128:(m + 1) * 128],
                                         xt[:, j, :],
                                         start=(j == 0), stop=(j == NJ - 1))
                    with nc.allow_low_precision(reason="bf16 activations"):
                        nc.vector.tensor_scalar_add(
                            dstT[:, m, sc * 512:(sc + 1) * 512], pp,
                            b_sb[:, m:m + 1])
            else:
                for st4 in range(4):
                    pp = psums.tile([128, 256], f32, tag="sm")
                    for j in range(NJ):
                        nc.tensor.matmul(pp, xt[:, j, st4 * 128:(st4 + 1) * 128],
                                         wv_sb[:, j, :],
                                         start=(j == 0), stop=(j == NJ - 1))
                    kt = sc * 4 + st4
                    nc.vector.tensor_copy(
                        out=vha[:, kt, :, :],
                        in_=pp.rearrange("p (h d) -> p h d", h=HPC))

        for sc in range(4):
            load_project(qt_d, "q", sc)
            load_project(kt_d, "k", sc)
            load_project(vt_d, "v", sc)

        # ---- phase 2: paired heads ----
        # Each stream-phase interleaves (a)-scores of one head with
        # (b)-scores of the opposite-parity head: lhsT row-groups alternate
        # (0-63 vs 64-127) so LDWEIGHTS pulls ahead and the PE streams,
        # and ACT always has (b)-work while (a) waits on expT reuse.
        rshs = {}
        stgs = {}

        def ab_chunk(ha, hb, t):
            """One q/k-block step for both streams, matmuls interleaved so
            consecutive PE ops alternate row-groups (64-partition heads of
            opposite parity) and LDWEIGHTS can pull ahead."""
            pa, hpa = (ha % 2) * 64, ha // 2
            pb, hpb = (hb % 2) * 64, hb // 2
            rsh = rshs[hb]
            rsp = small.tile([128, 2], f32, tag="rsp")
            wnorm = wstp2.tile([128, 2048], f32, tag="wnormbig")
            wraw = wstp2.tile([128, 1024], f32, tag="wraw")
            for half in range(2):
                psA = psum.tile([128, 1024], f32, tag="med")
                psB = psum.tile([128, 1024], f32, tag="med")
                for c in range(2):
                    q0 = half * 2 + c
                    nc.tensor.matmul(
                        psA[:, c * 512:(c + 1) * 512],
                        khT[pa:pa + 64, hpa, t * 128:(t + 1) * 128],
                        qhT[pa:pa + 64, hpa, q0 * 512:(q0 + 1) * 512],
                        start=True, stop=True)
                    nc.tensor.matmul(
                        psB[:, c * 512:(c + 1) * 512],
                        qhT[pb:pb + 64, hpb, t * 128:(t + 1) * 128],
                        khT[pb:pb + 64, hpb, q0 * 512:(q0 + 1) * 512],
                        start=True, stop=True)
                nc.scalar.activation(
                    out=expT[:, t, half * 1024:(half + 1) * 1024], in_=psA,
                    func=ACT_EXP)
                nc.scalar.activation(
                    out=wraw if half == 0 else wnorm[:, 1024:2048],
                    in_=psB, func=ACT_EXP, accum_out=rsp[:, half:half + 1])
            nc.vector.tensor_add(rsh[:, t:t + 1], rsp[:, 0:1], rsp[:, 1:2])
            nc.gpsimd.normalize_recip(wnorm[:, 0:1024], wraw, rsh[:, t:t + 1])
            with nc.allow_low_precision(reason="bf16 path"):
                nc.vector.tensor_scalar_mul(wnorm[:, 1024:2048],
                                            wnorm[:, 1024:2048],
                                            rsh[:, t:t + 1])
            nc.sync.dma_start(out=w_out[hb, t * 128:(t + 1) * 128, :], in_=wnorm)

        def stream_phase(ha, hb):
            rshs[hb] = small.tile([128, NT], f32, tag="rsh", name="rsh")
            for t in range(NT):
                ab_chunk(ha, hb, t)

        def attn_mms(h):
            out = []
            for qt in range(4):
                psAt = psums.tile([64, 512], f32, tag="sm")
                for kt in range(NT):
                    nc.tensor.matmul(
                        psAt, vha[:, kt, h, :],
                        expT[:, kt, qt * 512:(qt + 1) * 512],
                        start=(kt == 0), stop=(kt == NT - 1))
                stg = stage.tile([64, 512], bf16, tag="stg")
                nc.vector.tensor_copy(out=stg, in_=psAt)
                out.append(stg)
            stgs[h] = out

        def attn_norm(h):
            # broadcast 1/rowsum along q (free axis): transpose rsh on PE,
            # bounce through DRAM, partition-broadcast load with bf16 cast
            p0, hp = (h % 2) * 64, h // 2
            psr = psums.tile([NT, 128], f32, tag="sm")
            nc.tensor.transpose(psr, rshs[h], ident)
            rcT = small.tile([NT, 128], f32, tag="rcT")
            nc.vector.tensor_copy(out=rcT, in_=psr)
            drh = dram.tile([NT, 128], f32, tag="drh")
            nc.sync.dma_start(out=drh, in_=rcT)
            bc = bcp.tile([64, S], bf16, tag="bc")
            nc.gpsimd.dma_start(
                out=bc,
                in_=drh.rearrange("t (o p) -> o (t p)", o=1).to_broadcast([64, S]))
            for qt in range(4):
                nc.vector.tensor_mul(
                    attnT[p0:p0 + 64, hp, qt * 512:(qt + 1) * 512],
                    stgs[h][qt], bc[:, qt * 512:(qt + 1) * 512])

        stream_phase(0, 1)   # expT_0, rsh_1
        attn_mms(0)          # runs alongside next phase
        stream_phase(1, 0)   # expT_1 (waits on attn_0 reads), rsh_0
        attn_norm(0)
        attn_mms(1)
        attn_norm(1)
        stream_phase(2, 3)
        attn_mms(2)
        stream_phase(3, 2)
        attn_norm(2)
        attn_mms(3)
        attn_norm(3)

        # ---- phase 3: output projection (partial, no bias) ----
        for st in range(NT):
            psO = psum.tile([128, 1024], f32, tag="med")
            for oc in range(2):
                for m in range(2):
                    nc.tensor.matmul(
                        psO[:, oc * 512:(oc + 1) * 512],
                        attnT[:, m, st * 128:(st + 1) * 128],
                        wo_sb[:, m, oc * 512:(oc + 1) * 512],
                        start=(m == 0), stop=(m == 1))
            ost = ostp.tile([128, DIM], f32, tag="ost")
            nc.vector.tensor_copy(out=ost, in_=psO)
            nc.sync.dma_start(out=o_out[st * 128:(st + 1) * 128, :], in_=ost)

    nc.compile()
    return nc


def _prep_core_inputs(c, q, k, v, Wq, bq, Wk, bk, Wv, Wo, ident):
    b, g = divmod(c, HPC)
    hs = g * HPC
    sl = slice(hs, hs + HPC)
    return {
        "qT": np.ascontiguousarray(q[b].T.astype(ml_dtypes.bfloat16)),
        "kT": np.ascontiguousarray(k[b].T.astype(ml_dtypes.bfloat16)),
        "vT": np.ascontiguousarray(v[b].T.astype(ml_dtypes.bfloat16)),
        "wq": np.ascontiguousarray(
            (Wq[sl] * SCALE).transpose(1, 0, 2).reshape(DIM, HPC * DK)
            .astype(ml_dtypes.bfloat16)),
        "wk": np.ascontiguousarray(
            Wk[sl].transpose(1, 0, 2).reshape(DIM, HPC * DK)
            .astype(ml_dtypes.bfloat16)),
        "wv": np.ascontiguousarray(
            Wv[sl].transpose(1, 0, 2).reshape(DIM, HPC * DV)
            .astype(ml_dtypes.bfloat16)),
        "bq": np.ascontiguousarray(
            (bq[sl] * SCALE).reshape(HPC * DK).reshape(2, 128).T.astype(np.float32)),
        "bk": np.ascontiguousarray(
            bk[sl].reshape(HPC * DK).reshape(2, 128).T.astype(np.float32)),
        "wo": np.ascontiguousarray(
            Wo[g * 256:(g + 1) * 256].astype(ml_dtypes.bfloat16)),
        "ident": ident,
    }


def kernel(q, k, v, Wq, bq, Wk, bk, Wv, bv, Wo, bo):
    from concourse.bass_utils import run_bass_kernel_spmd

    q, k, v = (np.asarray(x, np.float32) for x in (q, k, v))
    Wq, bq, Wk, bk, Wv, bv, Wo, bo = (
        np.asarray(x, np.float32) for x in (Wq, bq, Wk, bk, Wv, bv, Wo, bo))

    if "nc" not in _cache:
        _cache["nc"] = _build()
    nc = _cache["nc"]

    ident = np.eye(128, dtype=np.float32)
    in_maps = [_prep_core_inputs(c, q, k, v, Wq, bq, Wk, bk, Wv, Wo, ident)
               for c in range(NCORES)]
    res = run_bass_kernel_spmd(nc, in_maps, core_ids=list(range(NCORES))).results

    weights = np.empty((B, H, S, S), np.float32)
    out = np.empty((B, S, DIM), np.float32)
    extra = (bv.reshape(DIM) @ Wo + bo).astype(np.float32)  # bv folds via softmax sum=1
    for b in range(B):
        acc = None
        for g in range(HPC):
            r = res[b * HPC + g]
            weights[b, g * HPC:(g + 1) * HPC] = r["w_out"]
            acc = r["o_out"] if acc is None else acc + r["o_out"]
        out[b] = acc + extra
    return out, weights


# revision 28
# speedup vs baseline: 1.0604x; 1.0504x over previous
"""MultiHeadAttention Trainium2 kernel (8-core sharded).

Reference computation (per batch b):
  qh = einsum('sd,hdk->hsk', q[b], Wq) + bq   (same k, v)
  scores = qh @ kh^T / sqrt(64); weights = softmax(scores)
  attn = weights @ vh; out = concat_heads(attn) @ Wo + bo
Returns (out [B,S,D], weights [B,H,S,S]).

Sharding: core c handles batch b = c//4 and heads [4g, 4g+4), g = c%4.
Each core computes its 4 heads' weights plus a partial output projection
(contracting only its heads' slice of Wo); the host sums the 4 partials
per batch and adds the bias terms (bv folds into a constant row because
softmax rows sum to 1: attn = attn_nobv + bv, so out gains bv_flat@Wo).
The host ships q/k/v pre-transposed ([D, S]) so the device needs no PE
transposes for the projections.

All matmuls run in bf16 with f32 PSUM accumulation (exp and softmax
normalization stay f32). Device pipeline per core:
  1. projections on PE from bf16 qT/kT/vT chunks:
     qhT/khT [dk-on-partitions, s], vh [k-on-partitions, dv].
  2. Paired stream phases: (a)-orientation scoresT chunks of one head
     interleaved matmul-by-matmul with (b)-orientation scores chunks of
     the opposite-parity head (row-groups alternate, keeping the PE
     array streaming). (a): exp (ACT) -> expT bf16 for attention.
     (b): exp+rowsum (ACT) -> normalize (GpSimd normalize_recip, which
     also writes back 1/rowsum) -> weights DMA (f32).
  3. attn: attnT_un = vh^T @ expT (PE, bf16); normalized along the
     free q axis with the (b) reciprocals via PE-mini-transpose ->
     DRAM bounce -> partition-broadcast load -> DVE multiply.
  4. output projection (PE, bf16) -> partial out DMA (f32).
"""

import numpy as np
import ml_dtypes

B, S, DIM, H, DK, DV = 2, 2048, 1024, 16, 64, 64
HPC = 4                     # heads per core
NCORES = 8
SCALE = 1.0 / np.sqrt(DV)   # folded into Wq/bq on host
NT = S // 128               # 16 s-tiles
NJ = DIM // 128             # 8 d-chunks

_cache = {}


def _build():
    import concourse.bass as bass
    import concourse.mybir as mybir
    import concourse.tile as tile
    from concourse import bacc
    from contextlib import ExitStack

    f32 = mybir.dt.float32
    f32r = mybir.dt.float32r
    bf16 = mybir.dt.bfloat16
    ACT_EXP = mybir.ActivationFunctionType.Exp

    nc = bacc.Bacc(None, target_bir_lowering=False)

    qt_d = nc.dram_tensor("qT", [DIM, S], bf16, kind="ExternalInput")
    kt_d = nc.dram_tensor("kT", [DIM, S], bf16, kind="ExternalInput")
    vt_d = nc.dram_tensor("vT", [DIM, S], bf16, kind="ExternalInput")
    wq_d = nc.dram_tensor("wq", [DIM, 256], bf16, kind="ExternalInput")
    wk_d = nc.dram_tensor("wk", [DIM, 256], bf16, kind="ExternalInput")
    wv_d = nc.dram_tensor("wv", [DIM, 256], bf16, kind="ExternalInput")
    bq_d = nc.dram_tensor("bq", [128, 2], f32, kind="ExternalInput")
    bk_d = nc.dram_tensor("bk", [128, 2], f32, kind="ExternalInput")
    wo_d = nc.dram_tensor("wo", [256, DIM], bf16, kind="ExternalInput")
    id_d = nc.dram_tensor("ident", [128, 128], f32, kind="ExternalInput")
    w_out = nc.dram_tensor("w_out", [HPC, S, S], f32, kind="ExternalOutput")
    o_out = nc.dram_tensor("o_out", [S, DIM], f32, kind="ExternalOutput")

    with tile.TileContext(nc) as tc, ExitStack() as ctx:
        singles = ctx.enter_context(tc.tile_pool(name="singles", bufs=1))
        persist = ctx.enter_context(tc.tile_pool(name="persist", bufs=1))
        qtp = ctx.enter_context(tc.tile_pool(name="qtp", bufs=1))
        wstp2 = ctx.enter_context(tc.tile_pool(name="wstp2", bufs=3))
        ostp = ctx.enter_context(tc.tile_pool(name="ostp", bufs=1))
        small = ctx.enter_context(tc.tile_pool(name="small", bufs=2))
        stage = ctx.enter_context(tc.tile_pool(name="stage", bufs=4))
        bcp = ctx.enter_context(tc.tile_pool(name="bcp", bufs=1))
        psum = ctx.enter_context(tc.tile_pool(name="ps", bufs=3, space="PSUM"))
        psums = ctx.enter_context(tc.tile_pool(name="pss", bufs=2, space="PSUM"))
        dram = ctx.enter_context(tc.tile_pool(name="drs", bufs=2, space="DRAM"))

        # ---- constants ----
        ident = singles.tile([128, 128], f32, tag="ident")
        nc.sync.dma_start(out=ident, in_=id_d[:, :])
        wq_sb = singles.tile([128, NJ, 256], bf16, tag="wq")
        wk_sb = singles.tile([128, NJ, 256], bf16, tag="wk")
        wv_sb = singles.tile([128, NJ, 256], bf16, tag="wv")
        nc.sync.dma_start(out=wq_sb, in_=wq_d.rearrange("(j p) c -> p j c", p=128))
        nc.sync.dma_start(out=wk_sb, in_=wk_d.rearrange("(j p) c -> p j c", p=128))
        nc.sync.dma_start(out=wv_sb, in_=wv_d.rearrange("(j p) c -> p j c", p=128))
        wo_sb = singles.tile([128, 2, DIM], bf16, tag="wo")
        nc.sync.dma_start(out=wo_sb, in_=wo_d.rearrange("(m p) c -> p m c", p=128))
        bq_sb = singles.tile([128, 2], f32, tag="bq")
        bk_sb = singles.tile([128, 2], f32, tag="bk")
        nc.sync.dma_start(out=bq_sb, in_=bq_d[:, :])
        nc.sync.dma_start(out=bk_sb, in_=bk_d[:, :])

        # ---- persistent activations ----
        qhT = persist.tile([128, 2, S], bf16, tag="qhT")   # [(h%2)*64+dk, h//2, s]
        khT = persist.tile([128, 2, S], bf16, tag="khT")
        vha = persist.tile([128, NT, HPC, 64], bf16, tag="vha")  # [k%128, kt, h, dv]
        expT = persist.tile([128, NT, S], bf16, tag="expT")      # per-head reuse
        attnT = persist.tile([128, 2, S], bf16, tag="attnT")     # [(h%2)*64+dv, h//2, q]

        # ---- phase 1: load transposed chunks, project ----
        def load_project(src_d, which, sc):
            """One 512-col slab of qT/kT/vT: load, project."""
            xt = qtp.tile([128, NJ, 512], bf16, tag="xt", bufs=2)  # [d%128, j, s-in-slab]
            nc.sync.dma_start(
                out=xt,
                in_=src_d.rearrange("(j p) s -> p j s", p=128)[
                    :, :, sc * 512:(sc + 1) * 512])
            if which == "q" or which == "k":
                w_sb = wq_sb if which == "q" else wk_sb
                b_sb = bq_sb if which == "q" else bk_sb
                dstT = qhT if which == "q" else khT
                for m in range(2):
                    pp = psums.tile([128, 512], f32, tag="sm")
                    for j in range(NJ):
                        nc.tensor.matmul(pp, w_sb[:, j, m * 128:(m + 1) * 128],
                                         xt[:, j, :],
                                         start=(j == 0), stop=(j == NJ - 1))
                    with nc.allow_low_precision(reason="bf16 activations"):
                        nc.vector.tensor_scalar_add(
                            dstT[:, m, sc * 512:(sc + 1) * 512], pp,
                            b_sb[:, m:m + 1])
            else:
                for st4 in range(4):
                    pp = psums.tile([128, 256], f32, tag="sm")
                    for j in range(NJ):
                        nc.tensor.matmul(pp, xt[:, j, st4 * 128:(st4 + 1) * 128],
                                         wv_sb[:, j, :],
                                         start=(j == 0), stop=(j == NJ - 1))
                    kt = sc * 4 + st4
                    nc.vector.tensor_copy(
                        out=vha[:, kt, :, :],
                        in_=pp.rearrange("p (h d) -> p h d", h=HPC))

        for sc in range(4):
            load_project(qt_d, "q", sc)
            load_project(kt_d, "k", sc)
            load_project(vt_d, "v", sc)

        # ---- phase 2: paired heads ----
        # Each stream-phase interleaves (a)-scores of one head with
        # (b)-scores of the opposite-parity head: lhsT row-groups alternate
        # (0-63 vs 64-127) so LDWEIGHTS pulls ahead and the PE streams,
        # and ACT always has (b)-work while (a) waits on expT reuse.
        rshs = {}
        stgs = {}

        def ab_chunk(ha, hb, t):
            """One q/k-block step for both streams, matmuls interleaved so
            consecutive PE ops alternate row-groups (64-partition heads of
            opposite parity) and LDWEIGHTS can pull ahead."""
            pa, hpa = (ha % 2) * 64, ha // 2
            pb, hpb = (hb % 2) * 64, hb // 2
            rsh = rshs[hb]
            rsp = small.tile([128, 2], f32, tag="rsp")
            wnorm = wstp2.tile([128, 2048], f32, tag="wnormbig")
            wraw = wstp2.tile([128, 1024], f32, tag="wraw", bufs=2)
            for half in range(2):
                psA = psum.tile([128, 1024], f32, tag="med")
                psB = psum.tile([128, 1024], f32, tag="med")
                for c in range(2):
                    q0 = half * 2 + c
                    nc.tensor.matmul(
                        psA[:, c * 512:(c + 1) * 512],
                        khT[pa:pa + 64, hpa, t * 128:(t + 1) * 128],
                        qhT[pa:pa + 64, hpa, q0 * 512:(q0 + 1) * 512],
                        start=True, stop=True)
                    nc.tensor.matmul(
                        psB[:, c * 512:(c + 1) * 512],
                        qhT[pb:pb + 64, hpb, t * 128:(t + 1) * 128],
                        khT[pb:pb + 64, hpb, q0 * 512:(q0 + 1) * 512],
                        start=True, stop=True)
                nc.scalar.activation(
                    out=expT[:, t, half * 1024:(half + 1) * 1024], in_=psA,
                    func=ACT_EXP)
                nc.scalar.activation(
                    out=wraw if half == 0 else wnorm[:, 1024:2048],
                    in_=psB, func=ACT_EXP, accum_out=rsp[:, half:half + 1])
            nc.vector.tensor_add(rsh[:, t:t + 1], rsp[:, 0:1], rsp[:, 1:2])
            nc.gpsimd.normalize_recip(wnorm[:, 0:1024], wraw, rsh[:, t:t + 1])
            with nc.allow_low_precision(reason="bf16 path"):
                nc.vector.tensor_scalar_mul(wnorm[:, 1024:2048],
                                            wnorm[:, 1024:2048],
                                            rsh[:, t:t + 1])
            nc.sync.dma_start(out=w_out[hb, t * 128:(t + 1) * 128, :], in_=wnorm)

        def stream_phase(ha, hb):
            rshs[hb] = small.tile([128, NT], f32, tag="rsh", name="rsh")
            for t in range(NT):
                ab_chunk(ha, hb, t)

        def attn_mms(h):
            out = []
            for qt in range(4):
                psAt = psums.tile([64, 512], f32, tag="sm")
                for kt in range(NT):
                    nc.tensor.matmul(
                        psAt, vha[:, kt, h, :],
                        expT[:, kt, qt * 512:(qt + 1) * 512],
                        start=(kt == 0), stop=(kt == NT - 1))
                stg = stage.tile([64, 512], bf16, tag="stg")
                nc.vector.tensor_copy(out=stg, in_=psAt)
                out.append(stg)
            stgs[h] = out

        def attn_norm(h):
            # broadcast 1/rowsum along q (free axis): transpose rsh on PE,
            # bounce through DRAM, partition-broadcast load with bf16 cast
            p0, hp = (h % 2) * 64, h // 2
            psr = psums.tile([NT, 128], f32, tag="sm")
            nc.tensor.transpose(psr, rshs[h], ident)
            rcT = small.tile([NT, 128], f32, tag="rcT")
            nc.vector.tensor_copy(out=rcT, in_=psr)
            drh = dram.tile([NT, 128], f32, tag="drh")
            nc.sync.dma_start(out=drh, in_=rcT)
            bc = bcp.tile([64, S], bf16, tag="bc")
            nc.gpsimd.dma_start(
                out=bc,
                in_=drh.rearrange("t (o p) -> o (t p)", o=1).to_broadcast([64, S]))
            for qt in range(4):
                nc.vector.tensor_mul(
                    attnT[p0:p0 + 64, hp, qt * 512:(qt + 1) * 512],
                    stgs[h][qt], bc[:, qt * 512:(qt + 1) * 512])

        stream_phase(0, 1)   # expT_0, rsh_1
        attn_mms(0)          # runs alongside next phase
        stream_phase(1, 0)   # expT_1 (waits on attn_0 reads), rsh_0
        attn_norm(0)
        attn_mms(1)
        attn_norm(1)
        stream_phase(2, 3)
        attn_mms(2)
        stream_phase(3, 2)
        attn_norm(2)
        attn_mms(3)
        attn_norm(3)

        # ---- phase 3: output projection (partial, no bias) ----
        for st in range(NT):
            psO = psum.tile([128, 1024], f32, tag="med")
            for oc in range(2):
                for m in range(2):
                    nc.tensor.matmul(
                        psO[:, oc * 512:(oc + 1) * 512],
                        attnT[:, m, st * 128:(st + 1) * 128],
                        wo_sb[:, m, oc * 512:(oc + 1) * 512],
                        start=(m == 0), stop=(m == 1))
            ost = ostp.tile([128, DIM], f32, tag="ost")
            nc.vector.tensor_copy(out=ost, in_=psO)
            nc.sync.dma_start(out=o_out[st * 128:(st + 1) * 128, :], in_=ost)

    nc.compile()
    return nc


def _prep_core_inputs(c, q, k, v, Wq, bq, Wk, bk, Wv, Wo, ident):
    b, g = divmod(c, HPC)
    hs = g * HPC
    sl = slice(hs, hs + HPC)
    return {
        "qT": np.ascontiguousarray(q[b].T.astype(ml_dtypes.bfloat16)),
        "kT": np.ascontiguousarray(k[b].T.astype(ml_dtypes.bfloat16)),
        "vT": np.ascontiguousarray(v[b].T.astype(ml_dtypes.bfloat16)),
        "wq": np.ascontiguousarray(
            (Wq[sl] * SCALE).transpose(1, 0, 2).reshape(DIM, HPC * DK)
            .astype(ml_dtypes.bfloat16)),
        "wk": np.ascontiguousarray(
            Wk[sl].transpose(1, 0, 2).reshape(DIM, HPC * DK)
            .astype(ml_dtypes.bfloat16)),
        "wv": np.ascontiguousarray(
            Wv[sl].transpose(1, 0, 2).reshape(DIM, HPC * DV)
            .astype(ml_dtypes.bfloat16)),
        "bq": np.ascontiguousarray(
            (bq[sl] * SCALE).reshape(HPC * DK).reshape(2, 128).T.astype(np.float32)),
        "bk": np.ascontiguousarray(
            bk[sl].reshape(HPC * DK).reshape(2, 128).T.astype(np.float32)),
        "wo": np.ascontiguousarray(
            Wo[g * 256:(g + 1) * 256].astype(ml_dtypes.bfloat16)),
        "ident": ident,
    }


def kernel(q, k, v, Wq, bq, Wk, bk, Wv, bv, Wo, bo):
    from concourse.bass_utils import run_bass_kernel_spmd

    q, k, v = (np.asarray(x, np.float32) for x in (q, k, v))
    Wq, bq, Wk, bk, Wv, bv, Wo, bo = (
        np.asarray(x, np.float32) for x in (Wq, bq, Wk, bk, Wv, bv, Wo, bo))

    if "nc" not in _cache:
        _cache["nc"] = _build()
    nc = _cache["nc"]

    ident = np.eye(128, dtype=np.float32)
    in_maps = [_prep_core_inputs(c, q, k, v, Wq, bq, Wk, bk, Wv, Wo, ident)
               for c in range(NCORES)]
    res = run_bass_kernel_spmd(nc, in_maps, core_ids=list(range(NCORES))).results

    weights = np.empty((B, H, S, S), np.float32)
    out = np.empty((B, S, DIM), np.float32)
    extra = (bv.reshape(DIM) @ Wo + bo).astype(np.float32)  # bv folds via softmax sum=1
    for b in range(B):
        acc = None
        for g in range(HPC):
            r = res[b * HPC + g]
            weights[b, g * HPC:(g + 1) * HPC] = r["w_out"]
            acc = r["o_out"] if acc is None else acc + r["o_out"]
        out[b] = acc + extra
    return out, weights
